# revision 17
# baseline (speedup 1.0000x reference)
"""CBAM kernel for Trainium2, 8-way batch-parallel SPMD — v5.

out = x^2 * (att_c[b,c] + sigmoid(conv(spatial_stats))[b,l]) per the CBAM
reference (out = x*ca + x*sa with ca = x*att_c, sa = x*sigmoid(conv)).

Layout: per core 4 batches; batch x[4096, 256] lives in SBUF as one
[128, 8192] bf16 tensor with partition p = l // 32, free col = 256*(l % 32)
+ c (so l = 32p + r).  Output stored bf16 (host upcasts).

v5 vs v4:
  - F = att⊕sig is built on the PE: per 1024-col chunk, two identity
    matmuls with stride-0 broadcast rhs APs accumulate sig (bcast over c)
    and att (bcast over r) into PSUM; ACT copies chunks out as bf16.
    This removes the F-add from the DVE entirely.
  - h2/h2r/cmf moved to GpSimd — the DVE instruction queue contains only
    tree ops + final multiplies, so nothing ever head-of-line-blocks it.
  - Issue order per batch: trees -> chan-max -> conv path (sigb/sig) ->
    cross-partition/MLP -> squares last; F chunks + mul + store issued
    after the NEXT batch's trees so DVE never waits on att.
  - Dependency-free ACT table warm op at t=0.

Engine split per batch:
  PE   : channel-sum matmuls, MLP, stats transposes, banded-Toeplitz conv,
         F build (broadcast identity matmuls)
  ACT  : squares, sigmoids, F chunk copies (PSUM->SBUF bf16)
  DVE  : spatial sum/max fold trees (bf16 2x), channel-max fold,
         block transpose + in-block reduce, final multiplies (2x)
  POOL : input cast-DMAs (SWDGE), h2/h2r/cmf tiny ops
"""

import numpy as np
from contextlib import ExitStack

import ml_dtypes

import concourse.bacc as bacc
import concourse.bass as bass
import concourse.tile as tile
import concourse.mybir as mybir
from concourse.bass_utils import run_bass_kernel_spmd

AF = mybir.ActivationFunctionType
ALU = mybir.AluOpType
AX = mybir.AxisListType
FP32 = mybir.dt.float32
BF16 = mybir.dt.bfloat16

N_CORES = 8
B_FULL = 32
NB = B_FULL // N_CORES  # batches per core = 4
L = 4096
C = 256
HID = 16
HB = HID + 1
P = 128
R = 32   # L-rows per partition (l = 32*p + r)
RH = 16  # rows per half
HC = RH * C  # 4096 free cols per half
FCH = 512    # F-build chunk cols (2 r-groups, one PSUM bank)
NCH = R * C // FCH  # 8 chunks per batch

_CACHE: dict = {}


def _fold_tree(nc, pool, src_ap, nr, w0, out_ap, name, op, f32_from,
               red_from=8):
    """Fold [128, (nr, w0)] down to [128, nr] into out_ap.

    Binary tt folds until width red_from, then one tensor_reduce.
    Levels with width >= f32_from stay bf16 (2x DVE rate).
    """
    cur = src_ap
    w = w0
    lvl = 0
    while w > red_from:
        hw = w // 2
        dt = BF16 if hw >= f32_from else FP32
        out = pool.tile([P, nr * hw], dt, tag=f"{name}{lvl}",
                        name=f"{name}{lvl}")[:]
        cv = cur.rearrange("p (r c) -> p r c", c=w)
        ov = out.rearrange("p (r c) -> p r c", c=hw)
        nc.vector.tensor_tensor(ov, cv[:, :, 0:hw], cv[:, :, hw:w], op)
        cur = out
        w = hw
        lvl += 1
    nc.vector.tensor_reduce(out_ap,
                            cur.rearrange("p (r c) -> p r c", c=w),
                            axis=AX.X, op=op)


def _build_body(ctx: ExitStack, tc, out_d, x_d, id_d, idb_d):
    nc = tc.nc

    const = ctx.enter_context(tc.tile_pool(name="const", bufs=1))
    xpool = ctx.enter_context(tc.tile_pool(name="x", bufs=1))
    opool = ctx.enter_context(tc.tile_pool(name="outb", bufs=1))
    fbig = ctx.enter_context(tc.tile_pool(name="fbig", bufs=1))
    fpool = ctx.enter_context(tc.tile_pool(name="fold", bufs=1))
    spool = ctx.enter_context(tc.tile_pool(name="stats", bufs=2))
    apool = ctx.enter_context(tc.tile_pool(name="att", bufs=2))
    pacc = ctx.enter_context(tc.tile_pool(name="pacc", bufs=2, space="PSUM"))
    pwork = ctx.enter_context(tc.tile_pool(name="pwork", bufs=3, space="PSUM"))
    pF = ctx.enter_context(tc.tile_pool(name="pF", bufs=2, space="PSUM"))

    # dependency-free ACT table warm: loads the sigmoid set (which also
    # contains Square/Copy/Relu) before any real ACT op
    warm = spool.tile([1, 1], FP32, tag="warm")
    nc.vector.memset(warm[:], 0.0)
    warm2 = spool.tile([1, 1], FP32, tag="warm2")
    nc.scalar.activation(warm2[:], warm[:], AF.Sigmoid)

    # all params packed into two wide tensors -> two fast HWDGE DMAs
    # (nine separate tiny const DMAs starved behind the 16MB input flood:
    # the 256B redcol DMA was measured completing at 52us, gating the PE)
    cf = const.tile([P, 739], FP32)
    nc.sync.dma_start(cf[:], id_d[:])
    cb = const.tile([P, C], BF16)
    nc.sync.dma_start(cb[:], idb_d[:])
    w1 = cf[:, 0:2 * HB]
    w2b = cf[0:HB, 34:34 + C]
    b1 = cf[0:HB, 290:291]
    convA = cf[0:R, 291:291 + 3 * R]
    convM = cf[0:R, 387:387 + 3 * R]
    ones = cf[0:HB, 483:483 + P]
    ident = cf[:, 611:611 + P]
    redcol = cb[:, 0:1]
    identb = cb[:, 1:1 + P]

    # ---- prefetch all 8 half-batches (SWDGE cast fp32 -> bf16) ----
    xb = []
    for b in range(NB):
        xt = xpool.tile([P, R * C], BF16, tag=f"x{b}", name=f"x{b}")
        xd = x_d[b, :, :].rearrange("(p r) c -> p (r c)", p=P)
        for h in range(2):
            nc.gpsimd.dma_start(xt[:, HC * h:HC * (h + 1)],
                                xd[:, HC * h:HC * (h + 1)])
        xb.append(xt)

    def compute_stats(b):
        x = xb[b][:]
        pcs = pacc.tile([1, C], FP32, tag="pcs")
        sum_s = spool.tile([P, R], FP32, tag="sum_s")
        max_s = spool.tile([P, R], FP32, tag="max_s")

        # --- per-half: chan-sum matmuls + spatial sum/max trees ---
        for h in range(2):
            for r in range(RH):
                rr = RH * h + r
                nc.tensor.matmul(pcs[:], redcol,
                                 x[:, C * rr:C * (rr + 1)],
                                 start=(rr == 0), stop=(rr == R - 1),
                                 skip_group_check=True)
            with nc.allow_low_precision("bf16 fold levels; tol 2e-2"):
                _fold_tree(nc, fpool, x[:, HC * h:HC * (h + 1)], RH, C,
                           sum_s[:, RH * h:RH * (h + 1)], f"sa{h}",
                           ALU.add, f32_from=8)
            _fold_tree(nc, fpool, x[:, HC * h:HC * (h + 1)], RH, C,
                       max_s[:, RH * h:RH * (h + 1)], f"sm{h}",
                       ALU.max, f32_from=2)

        # --- chan-max fold within partitions (DVE) ---
        mb = fpool.tile([P, R * C // 2], BF16, tag="mb", name="mb")
        mbf = spool.tile([P, C], FP32, tag="mbf")
        nc.vector.tensor_max(mb[:], x[:, 0:R * C // 2], x[:, R * C // 2:])
        w = R * C // 4
        while w > C:
            nc.vector.tensor_max(mb[:, 0:w], mb[:, 0:w], mb[:, w:2 * w])
            w //= 2
        nc.vector.tensor_max(mbf[:], mb[:, 0:C], mb[:, C:2 * C])

        # --- conv path first: it only needs sum_s/max_s, so sig (and the
        # F build) become ready before the MLP's DMA machinery finishes ---
        ptam = pwork.tile([R, 2 * P], FP32, tag="pwork")
        nc.tensor.transpose(ptam[:, 0:P], sum_s[:], ident)
        nc.tensor.transpose(ptam[:, P:2 * P], max_s[:], ident)
        stAM = spool.tile([R, 2 * P], FP32, tag="stAM")
        nc.scalar.activation(stAM[:], ptam[:], AF.Copy)
        stA = stAM[:, 0:P]
        stM = stAM[:, P:2 * P]

        pc = pwork.tile([R, P], FP32, tag="pwork")
        nc.tensor.matmul(pc[:], convA[:, 0:R], stA,
                         start=True, stop=False, skip_group_check=True)
        nc.tensor.matmul(pc[:, 1:P], convA[:, R:2 * R], stA[:, 0:P - 1],
                         start=False, stop=False, skip_group_check=True)
        nc.tensor.matmul(pc[:, 0:P - 1], convA[0:3, 2 * R:3 * R],
                         stA[0:3, 1:P],
                         start=False, stop=False, skip_group_check=True)
        nc.tensor.matmul(pc[:], convM[:, 0:R], stM,
                         start=False, stop=False, skip_group_check=True)
        nc.tensor.matmul(pc[:, 1:P], convM[:, R:2 * R], stM[:, 0:P - 1],
                         start=False, stop=False, skip_group_check=True)
        nc.tensor.matmul(pc[:, 0:P - 1], convM[0:3, 2 * R:3 * R],
                         stM[0:3, 1:P],
                         start=False, stop=True, skip_group_check=True)
        sigb = spool.tile([R, P], FP32, tag="sigb")
        nc.scalar.activation(sigb[:], pc[:], AF.Sigmoid)
        psg = pwork.tile([P, R], FP32, tag="pwork")
        nc.tensor.transpose(psg[:], sigb[:], ident[0:R, 0:R])
        sig = apool.tile([P, R], BF16, tag="sig")
        nc.scalar.activation(sig[:], psg[:], AF.Copy)

        # --- cross-partition chan-max + MLP -> att ---
        bt = spool.tile([P, C], FP32, tag="bt")
        nc.vector.transpose(bt[:], mbf[:])
        red = spool.tile([P, 8], FP32, tag="red")
        nc.vector.tensor_reduce(red[:],
                                bt[:].rearrange("p (bj s) -> p bj s", s=32),
                                axis=AX.X, op=ALU.max)
        cm32 = spool.tile([32, 32], FP32, tag="cm32")
        for a in range(4):
            nc.scalar.dma_start(cm32[:, 8 * a:8 * (a + 1)],
                                red[32 * a:32 * (a + 1), :])
        cmf = spool.tile([32, 8], FP32, tag="cmf")
        nc.vector.tensor_reduce(cmf[:],
                                cm32[:].rearrange("r (a bj) -> r bj a", a=4),
                                axis=AX.X, op=ALU.max)

        stats_cm = spool.tile([P, 4], FP32, tag="stats_cm")
        avg_row = spool.tile([1, C], FP32, tag="avg_row")
        nc.scalar.activation(avg_row[:], pcs[:], AF.Copy)
        for hh in range(2):
            nc.scalar.dma_start(stats_cm[:, 2 * hh:2 * hh + 1],
                                avg_row[0:1, P * hh:P * (hh + 1)])
        for bj in range(8):
            q = 32 * (bj % 4)
            nc.scalar.dma_start(stats_cm[q:q + 32, 2 * (bj // 4) + 1:
                                         2 * (bj // 4) + 2],
                                cmf[:, bj:bj + 1])

        ph = pwork.tile([HB, 2], FP32, tag="pwork")
        nc.tensor.matmul(ph[:], w1[:, 0:HB], stats_cm[:, 0:2],
                         start=True, stop=False, skip_group_check=True)
        nc.tensor.matmul(ph[:], w1[:, HB:2 * HB], stats_cm[:, 2:4],
                         start=False, stop=True, skip_group_check=True)
        hsb = spool.tile([HB, 2], FP32, tag="hsb")
        nc.scalar.activation(hsb[:], ph[:], AF.Relu, bias=b1)
        # h2 = hsb0 + hsb1 via Relu(x + bias) (both operands are >= 0);
        # h2r replicates it across 128 cols via a stride-0 broadcast read
        h2 = spool.tile([HB, 1], FP32, tag="h2")
        nc.scalar.activation(h2[:], hsb[:, 0:1], AF.Relu, bias=hsb[:, 1:2])
        h2r = spool.tile([HB, P], FP32, tag="h2r")
        nc.scalar.activation(h2r[:], h2[:].broadcast_to([HB, P]), AF.Relu)
        po = pwork.tile([P, C], FP32, tag="pwork")
        nc.tensor.matmul(po[:], h2r[:], w2b, start=True, stop=True,
                         skip_group_check=True)
        att = apool.tile([P, C], BF16, tag="att")
        nc.scalar.activation(att[:], po[:], AF.Sigmoid)

        # --- squares last: bulk ACT work the scheduler can slot anywhere ---
        ob = opool.tile([P, R * C], BF16, tag=f"ob{b % 2}", name=f"ob{b % 2}")
        for h in range(2):
            nc.scalar.activation(ob[:, HC * h:HC * (h + 1)],
                                 x[:, HC * h:HC * (h + 1)], AF.Square)

        return ob, att, sig

    def finals(b, ob, att, sig):
        """PE F-build chunks -> ACT bf16 copies -> DVE mul -> store."""
        F = fbig.tile([P, R * C], BF16, tag=f"F{b % 2}", name=f"F{b % 2}")
        RG = FCH // C  # r-groups per chunk
        att_b2 = att[:].unsqueeze(1).broadcast_to([P, 2, C])
        for g in range(NCH):
            psf = pF.tile([P, FCH], FP32, tag="pF", name="psf")
            pv = psf[:].rearrange("p (r c) -> p r c", c=C)
            sig_b = sig[:, RG * g:RG * (g + 1)].unsqueeze(2) \
                                               .broadcast_to([P, 2, C])
            nc.tensor.matmul(pv, identb, sig_b,
                             start=True, stop=False, skip_group_check=True)
            nc.tensor.matmul(pv, identb, att_b2,
                             start=False, stop=True, skip_group_check=True)
            nc.scalar.activation(F[:, FCH * g:FCH * (g + 1)], psf[:],
                                 AF.Copy)
        nq = 4 if b == NB - 1 else 2
        qc = R * C // nq
        for h in range(nq):
            obh = ob[:, qc * h:qc * (h + 1)]
            with nc.allow_low_precision("bf16 attention factors; tol 2e-2"):
                nc.vector.tensor_tensor(obh, F[:, qc * h:qc * (h + 1)], obh,
                                        ALU.mult)
            nc.sync.dma_start(
                out_d[b, :, :].rearrange("(p r) c -> p (r c)",
                                         p=P)[:, qc * h:qc * (h + 1)], obh)

    prev = None
    for b in range(NB):
        cur = (b, *compute_stats(b))
        if prev is not None:
            finals(*prev)
        prev = cur
    finals(*prev)


def _build_nc():
    nc = bacc.Bacc("TRN2", target_bir_lowering=False, debug=False,
                   enable_asserts=False, num_devices=N_CORES)
    x_d = nc.dram_tensor("xb", [NB, L, C], FP32, kind="ExternalInput").ap()
    id_d = nc.dram_tensor("constsf", [P, 739], FP32,
                          kind="ExternalInput").ap()
    idb_d = nc.dram_tensor("constsb", [P, C], BF16,
                           kind="ExternalInput").ap()
    out_d = nc.dram_tensor("out", [NB, L, C], BF16, kind="ExternalOutput").ap()

    with tile.TileContext(nc) as tc:
        with ExitStack() as ctx:
            _build_body(ctx, tc, out_d, x_d, id_d, idb_d)
    nc.compile()
    return nc


def get_nc():
    if "nc" not in _CACHE:
        _CACHE["nc"] = _build_nc()
    return _CACHE["nc"]


def _prep_inputs(W1, b1, W2, b2, conv_w):
    """Host-side parameter preprocessing (shared across cores)."""
    W1 = np.asarray(W1, np.float32)
    W2 = np.asarray(W2, np.float32)
    b1 = np.asarray(b1, np.float32)
    b2 = np.asarray(b2, np.float32)
    conv_w = np.asarray(conv_w, np.float32)

    w1sb = np.zeros((P, 2 * HB), np.float32)
    for h in range(2):
        w1sb[:, HB * h:HB * h + HID] = W1[P * h:P * (h + 1), :]
    w2b = np.concatenate([W2, b2[None, :]], axis=0).astype(np.float32)
    b1col = np.concatenate([b1, [1.0]]).astype(np.float32).reshape(HB, 1)

    # Banded Toeplitz over two adjacent 32-blocks; avg band folds in the
    # 1/C spatial-mean scale (device computes raw channel sums).
    def band64(w):
        Wb = np.zeros((64, 64), np.float32)
        for i in range(64):
            for k in range(7):
                j = i + k - 3
                if 0 <= j < 64:
                    Wb[i, j] = w[k]
        return Wb

    def pack(Wb):
        cv = np.zeros((R, 3 * R), np.float32)
        cv[:, 0:R] = Wb[0:R, 0:R].T          # main band
        cv[:, R:2 * R] = Wb[R:2 * R, 0:R].T  # prev-column corner
        cv[0:3, 2 * R:3 * R] = Wb[0:R, R:2 * R].T[0:3, :]  # next-column
        return cv

    convA = pack(band64(conv_w[:, 0, 0] / C))
    convM = pack(band64(conv_w[:, 1, 0]))

    cf = np.zeros((P, 739), np.float32)
    cf[:, 0:2 * HB] = w1sb
    cf[0:HB, 34:34 + 256] = w2b
    cf[0:HB, 290] = b1col[:, 0]
    cf[0:R, 291:291 + 96] = convA
    cf[0:R, 387:387 + 96] = convM
    cf[0:HB, 483:483 + P] = 1.0
    cf[:, 611:611 + P] = np.eye(P, dtype=np.float32)
    cb = np.zeros((P, 256), ml_dtypes.bfloat16)
    cb[:, 0] = ml_dtypes.bfloat16(1.0 / L)
    cb[:, 1:1 + P] = np.eye(P, dtype=ml_dtypes.bfloat16)
    return {"constsf": cf, "constsb": np.ascontiguousarray(cb)}


def kernel(x, W1, b1, W2, b2, conv_w):
    nc = get_nc()
    x = np.asarray(x, np.float32)
    params = _prep_inputs(W1, b1, W2, b2, conv_w)
    in_maps = []
    for c in range(N_CORES):
        m = dict(params)
        m["xb"] = np.ascontiguousarray(x[NB * c:NB * (c + 1)])
        in_maps.append(m)
    _CACHE["last_in_maps"] = in_maps
    res = run_bass_kernel_spmd(nc, in_maps, list(range(N_CORES)))
    _CACHE["last_results"] = res
    return np.concatenate(
        [np.asarray(res.results[c]["out"]).astype(np.float32)
         for c in range(N_CORES)], axis=0)


# revision 18
# speedup vs baseline: 1.0389x; 1.0389x over previous
"""CBAM kernel for Trainium2, 8-way batch-parallel SPMD — v5.

out = x^2 * (att_c[b,c] + sigmoid(conv(spatial_stats))[b,l]) per the CBAM
reference (out = x*ca + x*sa with ca = x*att_c, sa = x*sigmoid(conv)).

Layout: per core 4 batches; batch x[4096, 256] lives in SBUF as one
[128, 8192] bf16 tensor with partition p = l // 32, free col = 256*(l % 32)
+ c (so l = 32p + r).  Output stored bf16 (host upcasts).

v5 vs v4:
  - F = att⊕sig is built on the PE: per 1024-col chunk, two identity
    matmuls with stride-0 broadcast rhs APs accumulate sig (bcast over c)
    and att (bcast over r) into PSUM; ACT copies chunks out as bf16.
    This removes the F-add from the DVE entirely.
  - h2/h2r/cmf moved to GpSimd — the DVE instruction queue contains only
    tree ops + final multiplies, so nothing ever head-of-line-blocks it.
  - Issue order per batch: trees -> chan-max -> conv path (sigb/sig) ->
    cross-partition/MLP -> squares last; F chunks + mul + store issued
    after the NEXT batch's trees so DVE never waits on att.
  - Dependency-free ACT table warm op at t=0.

Engine split per batch:
  PE   : channel-sum matmuls, MLP, stats transposes, banded-Toeplitz conv,
         F build (broadcast identity matmuls)
  ACT  : squares, sigmoids, F chunk copies (PSUM->SBUF bf16)
  DVE  : spatial sum/max fold trees (bf16 2x), channel-max fold,
         block transpose + in-block reduce, final multiplies (2x)
  POOL : input cast-DMAs (SWDGE), h2/h2r/cmf tiny ops
"""

import numpy as np
from contextlib import ExitStack

import ml_dtypes

import concourse.bacc as bacc
import concourse.bass as bass
import concourse.tile as tile
import concourse.mybir as mybir
from concourse.bass_utils import run_bass_kernel_spmd

AF = mybir.ActivationFunctionType
ALU = mybir.AluOpType
AX = mybir.AxisListType
FP32 = mybir.dt.float32
BF16 = mybir.dt.bfloat16

N_CORES = 8
B_FULL = 32
NB = B_FULL // N_CORES  # batches per core = 4
L = 4096
C = 256
HID = 16
HB = HID + 1
P = 128
R = 32   # L-rows per partition (l = 32*p + r)
RH = 16  # rows per half
HC = RH * C  # 4096 free cols per half
FCH = 1024   # F-build chunk cols (4 r-groups)
NCH = R * C // FCH  # 8 chunks per batch

_CACHE: dict = {}


def _fold_tree(nc, pool, src_ap, nr, w0, out_ap, name, op, f32_from,
               red_from=8):
    """Fold [128, (nr, w0)] down to [128, nr] into out_ap.

    Binary tt folds until width red_from, then one tensor_reduce.
    Levels with width >= f32_from stay bf16 (2x DVE rate).
    """
    cur = src_ap
    w = w0
    lvl = 0
    while w > red_from:
        hw = w // 2
        dt = BF16 if hw >= f32_from else FP32
        out = pool.tile([P, nr * hw], dt, tag=f"{name}{lvl}",
                        name=f"{name}{lvl}")[:]
        cv = cur.rearrange("p (r c) -> p r c", c=w)
        ov = out.rearrange("p (r c) -> p r c", c=hw)
        nc.vector.tensor_tensor(ov, cv[:, :, 0:hw], cv[:, :, hw:w], op)
        cur = out
        w = hw
        lvl += 1
    nc.vector.tensor_reduce(out_ap,
                            cur.rearrange("p (r c) -> p r c", c=w),
                            axis=AX.X, op=op)


def _build_body(ctx: ExitStack, tc, out_d, x_d, id_d, idb_d):
    nc = tc.nc

    const = ctx.enter_context(tc.tile_pool(name="const", bufs=1))
    xpool = ctx.enter_context(tc.tile_pool(name="x", bufs=1))
    opool = ctx.enter_context(tc.tile_pool(name="outb", bufs=1))
    fbig = ctx.enter_context(tc.tile_pool(name="fbig", bufs=1))
    fpool = ctx.enter_context(tc.tile_pool(name="fold", bufs=1))
    spool = ctx.enter_context(tc.tile_pool(name="stats", bufs=2))
    apool = ctx.enter_context(tc.tile_pool(name="att", bufs=2))
    pacc = ctx.enter_context(tc.tile_pool(name="pacc", bufs=2, space="PSUM"))
    pwork = ctx.enter_context(tc.tile_pool(name="pwork", bufs=2, space="PSUM"))
    pF = ctx.enter_context(tc.tile_pool(name="pF", bufs=2, space="PSUM"))

    # dependency-free ACT table warm: loads the sigmoid set (which also
    # contains Square/Copy/Relu) before any real ACT op
    warm = spool.tile([1, 1], FP32, tag="warm")
    nc.vector.memset(warm[:], 0.0)
    warm2 = spool.tile([1, 1], FP32, tag="warm2")
    nc.scalar.activation(warm2[:], warm[:], AF.Sigmoid)

    # all params packed into two wide tensors -> two fast HWDGE DMAs
    # (nine separate tiny const DMAs starved behind the 16MB input flood:
    # the 256B redcol DMA was measured completing at 52us, gating the PE)
    cf = const.tile([P, 739], FP32)
    nc.sync.dma_start(cf[:], id_d[:])
    cb = const.tile([P, C], BF16)
    nc.sync.dma_start(cb[:], idb_d[:])
    w1 = cf[:, 0:2 * HB]
    w2b = cf[0:HB, 34:34 + C]
    b1 = cf[0:HB, 290:291]
    convA = cf[0:R, 291:291 + 3 * R]
    convM = cf[0:R, 387:387 + 3 * R]
    ones = cf[0:HB, 483:483 + P]
    ident = cf[:, 611:611 + P]
    redcol = cb[:, 0:1]
    identb = cb[:, 1:1 + P]

    # ---- prefetch all 8 half-batches (SWDGE cast fp32 -> bf16) ----
    xb = []
    for b in range(NB):
        xt = xpool.tile([P, R * C], BF16, tag=f"x{b}", name=f"x{b}")
        xd = x_d[b, :, :].rearrange("(p r) c -> p (r c)", p=P)
        for h in range(2):
            nc.gpsimd.dma_start(xt[:, HC * h:HC * (h + 1)],
                                xd[:, HC * h:HC * (h + 1)])
        xb.append(xt)

    def compute_stats(b):
        x = xb[b][:]
        pcs = pacc.tile([1, 2 * C], FP32, tag="pcs")
        sum_s = spool.tile([P, R], FP32, tag="sum_s")
        max_s = spool.tile([P, R], FP32, tag="max_s")

        # --- per-half: chan-sum matmuls (paired 512-col slices; even r's
        # land in pcs[0:256], odd r's in pcs[256:512]) + sum/max trees ---
        for h in range(2):
            for r in range(RH // 2):
                rr = (RH // 2) * h + r
                nc.tensor.matmul(pcs[:], redcol,
                                 x[:, 2 * C * rr:2 * C * (rr + 1)],
                                 start=(rr == 0), stop=(rr == R // 2 - 1),
                                 skip_group_check=True)
            with nc.allow_low_precision("bf16 fold levels; tol 2e-2"):
                _fold_tree(nc, fpool, x[:, HC * h:HC * (h + 1)], RH, C,
                           sum_s[:, RH * h:RH * (h + 1)], f"sa{h}",
                           ALU.add, f32_from=8)
            _fold_tree(nc, fpool, x[:, HC * h:HC * (h + 1)], RH, C,
                       max_s[:, RH * h:RH * (h + 1)], f"sm{h}",
                       ALU.max, f32_from=2)

        # --- chan-max fold within partitions (DVE) ---
        mb = fpool.tile([P, R * C // 2], BF16, tag="mb", name="mb")
        mbf = spool.tile([P, C], FP32, tag="mbf")
        nc.vector.tensor_max(mb[:], x[:, 0:R * C // 2], x[:, R * C // 2:])
        w = R * C // 4
        while w > C:
            nc.vector.tensor_max(mb[:, 0:w], mb[:, 0:w], mb[:, w:2 * w])
            w //= 2
        nc.vector.tensor_max(mbf[:], mb[:, 0:C], mb[:, C:2 * C])

        # --- conv path first: it only needs sum_s/max_s, so sig (and the
        # F build) become ready before the MLP's DMA machinery finishes ---
        ptam = pwork.tile([R, 2 * P], FP32, tag="pwork")
        nc.tensor.transpose(ptam[:, 0:P], sum_s[:], ident)
        nc.tensor.transpose(ptam[:, P:2 * P], max_s[:], ident)
        stAM = spool.tile([R, 2 * P], FP32, tag="stAM")
        nc.scalar.activation(stAM[:], ptam[:], AF.Copy)
        stA = stAM[:, 0:P]
        stM = stAM[:, P:2 * P]

        pc = pwork.tile([R, P], FP32, tag="pwork")
        nc.tensor.matmul(pc[:], convA[:, 0:R], stA,
                         start=True, stop=False, skip_group_check=True)
        nc.tensor.matmul(pc[:, 1:P], convA[:, R:2 * R], stA[:, 0:P - 1],
                         start=False, stop=False, skip_group_check=True)
        nc.tensor.matmul(pc[:, 0:P - 1], convA[0:3, 2 * R:3 * R],
                         stA[0:3, 1:P],
                         start=False, stop=False, skip_group_check=True)
        nc.tensor.matmul(pc[:], convM[:, 0:R], stM,
                         start=False, stop=False, skip_group_check=True)
        nc.tensor.matmul(pc[:, 1:P], convM[:, R:2 * R], stM[:, 0:P - 1],
                         start=False, stop=False, skip_group_check=True)
        nc.tensor.matmul(pc[:, 0:P - 1], convM[0:3, 2 * R:3 * R],
                         stM[0:3, 1:P],
                         start=False, stop=True, skip_group_check=True)
        sigb = spool.tile([R, P], FP32, tag="sigb")
        nc.scalar.activation(sigb[:], pc[:], AF.Sigmoid)
        psg = pwork.tile([P, R], FP32, tag="pwork")
        nc.tensor.transpose(psg[:], sigb[:], ident[0:R, 0:R])
        sig = apool.tile([P, R], BF16, tag="sig")
        nc.scalar.activation(sig[:], psg[:], AF.Copy)

        # --- cross-partition chan-max + MLP -> att ---
        bt = spool.tile([P, C], FP32, tag="bt")
        nc.vector.transpose(bt[:], mbf[:])
        red = spool.tile([P, 8], FP32, tag="red")
        nc.vector.tensor_reduce(red[:],
                                bt[:].rearrange("p (bj s) -> p bj s", s=32),
                                axis=AX.X, op=ALU.max)
        cm32 = spool.tile([32, 32], FP32, tag="cm32")
        for a in range(4):
            nc.scalar.dma_start(cm32[:, 8 * a:8 * (a + 1)],
                                red[32 * a:32 * (a + 1), :])
        cmf = spool.tile([32, 8], FP32, tag="cmf")
        nc.vector.tensor_reduce(cmf[:],
                                cm32[:].rearrange("r (a bj) -> r bj a", a=4),
                                axis=AX.X, op=ALU.max)

        stats_cm = spool.tile([P, 4], FP32, tag="stats_cm")
        avg_row = spool.tile([1, 2 * C], FP32, tag="avg_row")
        nc.scalar.activation(avg_row[:], pcs[:], AF.Copy)
        avg2 = spool.tile([1, C], FP32, tag="avg2")
        nc.vector.tensor_add(avg2[:], avg_row[0:1, 0:C],
                             avg_row[0:1, C:2 * C])
        for hh in range(2):
            nc.scalar.dma_start(stats_cm[:, 2 * hh:2 * hh + 1],
                                avg2[0:1, P * hh:P * (hh + 1)])
        for bj in range(8):
            q = 32 * (bj % 4)
            nc.scalar.dma_start(stats_cm[q:q + 32, 2 * (bj // 4) + 1:
                                         2 * (bj // 4) + 2],
                                cmf[:, bj:bj + 1])

        ph = pwork.tile([HB, 2], FP32, tag="pwork")
        nc.tensor.matmul(ph[:], w1[:, 0:HB], stats_cm[:, 0:2],
                         start=True, stop=False, skip_group_check=True)
        nc.tensor.matmul(ph[:], w1[:, HB:2 * HB], stats_cm[:, 2:4],
                         start=False, stop=True, skip_group_check=True)
        hsb = spool.tile([HB, 2], FP32, tag="hsb")
        nc.scalar.activation(hsb[:], ph[:], AF.Relu, bias=b1)
        # h2 = hsb0 + hsb1 via Relu(x + bias) (both operands are >= 0);
        # h2r replicates it across 128 cols via a stride-0 broadcast read
        h2 = spool.tile([HB, 1], FP32, tag="h2")
        nc.scalar.activation(h2[:], hsb[:, 0:1], AF.Relu, bias=hsb[:, 1:2])
        h2r = spool.tile([HB, P], FP32, tag="h2r")
        nc.scalar.activation(h2r[:], h2[:].broadcast_to([HB, P]), AF.Relu)
        po = pwork.tile([P, C], FP32, tag="pwork")
        nc.tensor.matmul(po[:], h2r[:], w2b, start=True, stop=True,
                         skip_group_check=True)
        att = apool.tile([P, C], BF16, tag="att")
        nc.scalar.activation(att[:], po[:], AF.Sigmoid)

        # --- squares last: bulk ACT work the scheduler can slot anywhere ---
        ob = opool.tile([P, R * C], BF16, tag=f"ob{b % 2}", name=f"ob{b % 2}")
        for h in range(2):
            nc.scalar.activation(ob[:, HC * h:HC * (h + 1)],
                                 x[:, HC * h:HC * (h + 1)], AF.Square)

        return ob, att, sig

    def finals(b, ob, att, sig):
        """PE F-build chunks -> ACT bf16 copies -> DVE mul -> store."""
        F = fbig.tile([P, R * C], BF16, tag=f"F{b % 2}", name=f"F{b % 2}")
        RG = FCH // C  # r-groups per chunk
        att_b2 = att[:].unsqueeze(1).broadcast_to([P, 2, C])
        for g in range(NCH):
            psf = pF.tile([P, FCH], FP32, tag="pF", name="psf")
            for s in range(RG // 2):
                pv = psf[:, 2 * C * s:2 * C * (s + 1)] \
                    .rearrange("p (r c) -> p r c", c=C)
                r0 = RG * g + 2 * s
                sig_b = sig[:, r0:r0 + 2].unsqueeze(2) \
                                         .broadcast_to([P, 2, C])
                nc.tensor.matmul(pv, identb, sig_b,
                                 start=True, stop=False,
                                 skip_group_check=True)
                nc.tensor.matmul(pv, identb, att_b2,
                                 start=False, stop=True,
                                 skip_group_check=True)
            nc.scalar.activation(F[:, FCH * g:FCH * (g + 1)], psf[:],
                                 AF.Copy)
        nq = 4 if b == NB - 1 else 2
        qc = R * C // nq
        for h in range(nq):
            obh = ob[:, qc * h:qc * (h + 1)]
            with nc.allow_low_precision("bf16 attention factors; tol 2e-2"):
                nc.vector.tensor_tensor(obh, F[:, qc * h:qc * (h + 1)], obh,
                                        ALU.mult)
            nc.sync.dma_start(
                out_d[b, :, :].rearrange("(p r) c -> p (r c)",
                                         p=P)[:, qc * h:qc * (h + 1)], obh)

    prev = None
    for b in range(NB):
        cur = (b, *compute_stats(b))
        if prev is not None:
            finals(*prev)
        prev = cur
    finals(*prev)


def _build_nc():
    nc = bacc.Bacc("TRN2", target_bir_lowering=False, debug=False,
                   enable_asserts=False, num_devices=N_CORES)
    x_d = nc.dram_tensor("xb", [NB, L, C], FP32, kind="ExternalInput").ap()
    id_d = nc.dram_tensor("constsf", [P, 739], FP32,
                          kind="ExternalInput").ap()
    idb_d = nc.dram_tensor("constsb", [P, C], BF16,
                           kind="ExternalInput").ap()
    out_d = nc.dram_tensor("out", [NB, L, C], BF16, kind="ExternalOutput").ap()

    with tile.TileContext(nc) as tc:
        with ExitStack() as ctx:
            _build_body(ctx, tc, out_d, x_d, id_d, idb_d)
    nc.compile()
    return nc


def get_nc():
    if "nc" not in _CACHE:
        _CACHE["nc"] = _build_nc()
    return _CACHE["nc"]


def _prep_inputs(W1, b1, W2, b2, conv_w):
    """Host-side parameter preprocessing (shared across cores)."""
    W1 = np.asarray(W1, np.float32)
    W2 = np.asarray(W2, np.float32)
    b1 = np.asarray(b1, np.float32)
    b2 = np.asarray(b2, np.float32)
    conv_w = np.asarray(conv_w, np.float32)

    w1sb = np.zeros((P, 2 * HB), np.float32)
    for h in range(2):
        w1sb[:, HB * h:HB * h + HID] = W1[P * h:P * (h + 1), :]
    w2b = np.concatenate([W2, b2[None, :]], axis=0).astype(np.float32)
    b1col = np.concatenate([b1, [1.0]]).astype(np.float32).reshape(HB, 1)

    # Banded Toeplitz over two adjacent 32-blocks; avg band folds in the
    # 1/C spatial-mean scale (device computes raw channel sums).
    def band64(w):
        Wb = np.zeros((64, 64), np.float32)
        for i in range(64):
            for k in range(7):
                j = i + k - 3
                if 0 <= j < 64:
                    Wb[i, j] = w[k]
        return Wb

    def pack(Wb):
        cv = np.zeros((R, 3 * R), np.float32)
        cv[:, 0:R] = Wb[0:R, 0:R].T          # main band
        cv[:, R:2 * R] = Wb[R:2 * R, 0:R].T  # prev-column corner
        cv[0:3, 2 * R:3 * R] = Wb[0:R, R:2 * R].T[0:3, :]  # next-column
        return cv

    convA = pack(band64(conv_w[:, 0, 0] / C))
    convM = pack(band64(conv_w[:, 1, 0]))

    cf = np.zeros((P, 739), np.float32)
    cf[:, 0:2 * HB] = w1sb
    cf[0:HB, 34:34 + 256] = w2b
    cf[0:HB, 290] = b1col[:, 0]
    cf[0:R, 291:291 + 96] = convA
    cf[0:R, 387:387 + 96] = convM
    cf[0:HB, 483:483 + P] = 1.0
    cf[:, 611:611 + P] = np.eye(P, dtype=np.float32)
    cb = np.zeros((P, 256), ml_dtypes.bfloat16)
    cb[:, 0] = ml_dtypes.bfloat16(1.0 / L)
    cb[:, 1:1 + P] = np.eye(P, dtype=ml_dtypes.bfloat16)
    return {"constsf": cf, "constsb": np.ascontiguousarray(cb)}


def kernel(x, W1, b1, W2, b2, conv_w):
    nc = get_nc()
    x = np.asarray(x, np.float32)
    params = _prep_inputs(W1, b1, W2, b2, conv_w)
    in_maps = []
    for c in range(N_CORES):
        m = dict(params)
        m["xb"] = np.ascontiguousarray(x[NB * c:NB * (c + 1)])
        in_maps.append(m)
    _CACHE["last_in_maps"] = in_maps
    res = run_bass_kernel_spmd(nc, in_maps, list(range(N_CORES)))
    _CACHE["last_results"] = res
    return np.concatenate(
        [np.asarray(res.results[c]["out"]).astype(np.float32)
         for c in range(N_CORES)], axis=0)
